# revision 32
# baseline (speedup 1.0000x reference)
"""AttentionPooling TRN2 kernel v2: channel-on-partition layout, 8-core
data-parallel over the flattened (B*N) points.

Math (per point n with k=16 neighbors, C=512 channels):
  logits = x @ w_score.T            (per-channel attention logits)
  scores = softmax_k(logits)        (softmax over the k axis, per channel)
  pooled = sum_k x * scores
  y      = relu((pooled @ w_conv.T - mean) * gamma/sqrt(var+eps) + beta)

v1 kept (n,k) points on SBUF partitions, which forced the softmax-k
reductions onto the TensorE as G-matrix matmuls (~27us) and required PE
transposes before mm2 (~4us); PE busy was 296us of a 324us span.

v2 puts CHANNELS on partitions everywhere:
  - mm1 runs with the weights stationary (wstT [c,128d] blocks) and
    host-transposed fp16 xT as the moving operand, producing logits as
    [d-block, pts] in PSUM.  Same 218us of PE streaming, but the output
    layout now matches x's, so everything downstream is elementwise.
  - pts within a 512-pt window are ordered (k,n2): j = k*32 + n2.  The
    softmax-k reductions become log2(16)=4 pairwise tensor_tensor adds
    over contiguous half-slices -- fp16 2x_1P DVE mode, batched over all
    4 c-strips and 4 windows of a super-block in single instructions
    (58-cycle fixed cost amortized to nothing).
  - the whole k-reduction moves to the DVE (~131us) which was 60% idle,
    and mm2 consumes pooled [c,n] directly as its stationary operand --
    no transposes.  PE work drops to mm1 + mm2 + warmup = ~237us.
  - exp stays on ACT (146us), 1/esum uses the ACT Ln/Exp chain (exp is
    needed anyway and DVE reciprocal is 8.7ns/elem), relu moves to ACT.
  - engine budgets: PE ~237us, DVE ~213us, ACT ~176us, DMA 32MB/core
    (~90us) -- vs v1's 64MB (x was shipped twice in two layouts).
  - fp16 everywhere (x, weights, e, t, sums): mm1 in fp16 is 8x more
    accurate than v1's bf16 (sim: 6.6e-4 vs 1.6e-3 end-to-end rel err).
    fp8 DoubleRow mm1 was evaluated and rejected: e4m3 on both operands
    measures 2.8e-2 end-to-end (gate 2e-2), and the pair slots cannot
    carry error-compensation terms without giving back the 2x.
  - emission schedule: window c's 16 MMs, 4 exps, 1 mul; the per-sb tail
    (tree, recip, pooled-mul, mm2, relu+store) is spread over the next 4
    window slots so no engine stream ever stalls on a cross-engine dep.
"""
import numpy as np
import ml_dtypes

B, N, K, C, COUT = 4, 4096, 16, 512, 512
NCORES = 8
NROWS = B * N // NCORES           # 2048 n-points per core
NWIN = 64                         # 512-pt windows (32 n x 16 k) per core
NSB = 16                          # super-blocks of 4 windows (128 n)
BN_EPS = 1e-5

_cached = {}


def _patch_act_tables():
    """Pin Exp/Ln/Relu to the one table set that holds all three.

    The act-table-placement pass resolves each activation to a table set
    containing its function; exp's and ln's first-match sets differ, so the
    v2 profile showed 34 ACT_TABLE_LOADs (1.28us each) -- two per
    super-block around the Ln/Exp reciprocal, injected straight into the
    pooled->mm2 critical chain.  act_info.json's set 6
    (natural_log_exp_and_others) really contains exp+ln+relu; hiding those
    funcs from every OTHER set (list order, and therefore the positional
    act_func_set_id the pass emits, is unchanged) forces all three to
    resolve to set 6: one table load at kernel start, zero thereafter.
    The patch only narrows the pass's view; the runtime table actually
    loaded is the real set 6, so lowering stays correct.
    """
    import functools

    import concourse.bacc as bacc
    import concourse.hw_specs as hw_specs

    orig = hw_specs.get_activation_tables.__wrapped__

    @functools.cache
    def patched(arch):
        tabs = dict(orig(arch))
        keep = {"Exp", "Ln", "Relu"}
        out = {}
        for name, funcs in tabs.items():
            if name != "natural_log_exp_and_others":
                funcs = {f for f in funcs if f.name not in keep}
            out[name] = funcs
        return out

    bacc.get_activation_tables = patched


def _build():
    import concourse.bacc as bacc
    import concourse.mybir as mybir
    import concourse.tile as tile

    _patch_act_tables()
    F32, F16 = mybir.dt.float32, mybir.dt.float16
    ACT = mybir.ActivationFunctionType

    nc = bacc.Bacc("TRN2", target_bir_lowering=False, debug=False, num_devices=NCORES)
    xw = nc.dram_tensor("xw", [NWIN, 128, 4, 512], F16, kind="ExternalInput")
    wst = nc.dram_tensor("wst", [C, C], F16, kind="ExternalInput")      # w_score.T
    wc2 = nc.dram_tensor("wc2", [C, COUT], F16, kind="ExternalInput")   # (bn*w_conv).T
    bias2 = nc.dram_tensor("bias2", [128, 4], F16, kind="ExternalInput")
    y = nc.dram_tensor("y", [COUT, NROWS], F16, kind="ExternalOutput")

    with tile.TileContext(nc) as tc:
        with (
            tc.tile_pool(name="const", bufs=1) as cp,
            tc.tile_pool(name="xw", bufs=6) as xwp,
            tc.tile_pool(name="e", bufs=2) as ep,
            tc.tile_pool(name="t", bufs=2) as tp_,
            tc.tile_pool(name="r", bufs=1) as rp,
            tc.tile_pool(name="s", bufs=3) as sp,
            tc.tile_pool(name="o", bufs=2) as op_,
            tc.tile_pool(name="ps", bufs=8, space="PSUM") as psp,
        ):
            # ---- constants ----
            # wst strips interleave with window-0's strip DMAs below so the
            # first matmul is gated by 256KB of HBM traffic, not ~1MB; the
            # mm2-only constants (wc2/bias/ones) are deferred off the head's
            # HBM bandwidth entirely (first use is slot 7, ~25us in).
            wst_t = [cp.tile([128, C], F16, tag=f"wst{i}", name=f"wst{i}") for i in range(4)]
            wc2_t = [cp.tile([128, COUT], F16, tag=f"wc2{i}", name=f"wc2{i}") for i in range(4)]
            bias2_t = cp.tile([128, 4], F16, tag="bias2", name="bias2")

            def load_consts():
                for i in range(4):
                    nc.gpsimd.dma_start(wc2_t[i][:], wc2[128 * i:128 * (i + 1), :])
                nc.gpsimd.dma_start(bias2_t[:], bias2[:])

            # Dep-free warm-up: memset fp16 tile matmuls from t~0.3us keep the
            # PE busy while the first ~1MB of weights+x loads, so the HAM
            # clock-gate reaches 8/8 right as the real mm1 stream starts.
            wz0 = cp.tile([128, 128], F16, tag="wz0", name="wz0")
            nc.vector.memset(wz0[:], 0.25)
            warm_ps = psp.tile([128, C], F32, tag="pl", name="warm")
            import itertools
            warm_groups = itertools.cycle(range(4))

            def warm_burst(n):
                # dep-free matmuls on a memset tile: keep the PE's HAM
                # activity window busy through the head DMA ramp; each burst
                # is its own accumulation group on a distinct column slice
                g = next(warm_groups)
                for wmi in range(n):
                    nc.tensor.matmul(
                        warm_ps[:, 128 * g:128 * (g + 1)], wz0[:], wz0[:],
                        start=(wmi == 0), stop=(wmi == n - 1),
                    )

            warm_burst(24)
            # dep-free dummy Ln preloads the natural_log/exp ACT table set
            dume = cp.tile([1, 16], F32, tag="dume", name="dume")
            nc.vector.memset(dume[:], 1.0)
            dln = cp.tile([1, 16], F32, tag="dln", name="dln")
            nc.scalar.activation(dln[:], dume[:], ACT.Ln)

            xw_t = {}

            def issue_xw(w):
                t = xwp.tile([128, 4, 512], F16, tag="xw", name=f"xw{w}")
                nc.sync.dma_start(t[:], xw[w])
                xw_t[w] = t

            def issue_xw_strip(w, s):
                if w not in xw_t:
                    xw_t[w] = xwp.tile([128, 4, 512], F16, tag="xw",
                                       name=f"xw{w}")
                nc.sync.dma_start(xw_t[w][:, s, :], xw[w, :, s, :])

            state = {}
            sched = {}

            def at(slot, fn):
                sched.setdefault(slot, []).append(fn)

            def halve(dst, src, n):
                nc.vector.tensor_add(
                    dst[:], src[..., 0:n], src[..., n:2 * n]
                )

            def alloc_sums(sb):
                st = state[sb]
                st["es"] = sp.tile([128, 4, 4, 32], F16, tag="se",
                                   name=f"se{sb}")
                st["ts"] = sp.tile([128, 4, 4, 32], F16, tag="st",
                                   name=f"st{sb}")

            def mk_tree_sb(sb, key):
                # whole-super-block tree: 4 ops covering all 4 windows and
                # strips -- the 58-cycle DVE op overhead amortizes away
                st = state[sb]
                r1 = rp.tile([128, 4, 4, 256], F16, tag=f"r1{key}",
                             name=f"r1{key}{sb}")
                halve(r1, st[key], 256)
                r2 = rp.tile([128, 4, 4, 128], F16, tag=f"r2{key}",
                             name=f"r2{key}{sb}")
                halve(r2, r1, 128)
                r3 = rp.tile([128, 4, 4, 64], F16, tag=f"r3{key}",
                             name=f"r3{key}{sb}")
                halve(r3, r2, 64)
                halve(st["es" if key == "e" else "ts"], r3, 32)

            def mk_tree_win(sb, j, keys):
                # one window, all strips: spreads the reduction across the
                # window slots (e always; t only for the last super-block)
                st = state[sb]
                for key in keys:
                    src = st[key][:, :, j, :]
                    r1 = rp.tile([128, 4, 256], F16, tag=f"v1{key}",
                                 name=f"v1{key}{sb}_{j}")
                    nc.vector.tensor_add(r1[:], src[:, :, 0:256],
                                         src[:, :, 256:512])
                    r2 = rp.tile([128, 4, 128], F16, tag=f"v2{key}",
                                 name=f"v2{key}{sb}_{j}")
                    nc.vector.tensor_add(r2[:], r1[:, :, 0:128],
                                         r1[:, :, 128:256])
                    r3 = rp.tile([128, 4, 64], F16, tag=f"v3{key}",
                                 name=f"v3{key}{sb}_{j}")
                    nc.vector.tensor_add(r3[:], r2[:, :, 0:64],
                                         r2[:, :, 64:128])
                    dst = st["es" if key == "e" else "ts"]
                    nc.vector.tensor_add(dst[:, :, j, :], r3[:, :, 0:32],
                                         r3[:, :, 32:64])

            def mk_tree_strip(sb, j, db):
                # last-super-block tail: per-(window, strip) micro-trees so
                # strip db's sums complete while strip db+1's exp still runs
                st = state[sb]
                for key in ("e", "t"):
                    src = st[key][:, db, j, :]
                    r1 = rp.tile([128, 256], F16, tag=f"w1{key}",
                                 name=f"w1{key}{sb}_{j}_{db}")
                    nc.vector.tensor_add(r1[:], src[:, 0:256], src[:, 256:512])
                    r2 = rp.tile([128, 128], F16, tag=f"w2{key}",
                                 name=f"w2{key}{sb}_{j}_{db}")
                    nc.vector.tensor_add(r2[:], r1[:, 0:128], r1[:, 128:256])
                    r3 = rp.tile([128, 64], F16, tag=f"w3{key}",
                                 name=f"w3{key}{sb}_{j}_{db}")
                    nc.vector.tensor_add(r3[:], r2[:, 0:64], r2[:, 64:128])
                    dst = st["es" if key == "e" else "ts"]
                    nc.vector.tensor_add(dst[:, db, j, :], r3[:, 0:32],
                                         r3[:, 32:64])

            def mk_recip(sb):
                st = state[sb]
                lnt = sp.tile([128, 4, 4, 32], F32, tag="lnt", name=f"lnt{sb}")
                nc.scalar.activation(lnt[:], st["es"][:], ACT.Ln)
                inv = sp.tile([128, 4, 4, 32], F16, tag="inv", name=f"inv{sb}")
                nc.scalar.activation(inv[:], lnt[:], ACT.Exp, scale=-1.0)
                st["inv"] = inv

            def mk_pooled(sb):
                st = state[sb]
                pool_t = sp.tile([128, 4, 4, 32], F16, tag="pool",
                                 name=f"pool{sb}")
                nc.vector.tensor_mul(pool_t[:], st["ts"][:], st["inv"][:])
                st["pool"] = pool_t

            def mk_mm2(sb, fuse_out=False):
                # transposed mm2: out [d2-block, n] so the BN bias becomes a
                # per-partition ACT relu bias and the 216ns K=1 bias matmul
                # disappears; 16 MMs of 128-free stream at ~56ns each
                st = state[sb]
                py = psp.tile([128, 4, 128], F32, tag="pl", name=f"py{sb}")
                st["py"] = py
                for d2b in range(4):
                    for cs in range(4):
                        nc.tensor.matmul(
                            py[:, d2b, :],
                            wc2_t[cs][:, 128 * d2b:128 * (d2b + 1)],
                            st["pool"][:, cs],
                            start=(cs == 0), stop=(cs == 3),
                        )
                    if fuse_out:
                        # tail: overlap each d2-block's relu+store with the
                        # next block's matmuls
                        mk_out_block(sb, d2b)

            def mk_out_block(sb, d2b):
                st = state[sb]
                if d2b == 0:
                    st["yt"] = op_.tile([128, 4, 128], F16, tag="yt",
                                        name=f"yt{sb}")
                y_t = st["yt"]
                nc.scalar.activation(y_t[:, d2b, :], st["py"][:, d2b, :],
                                     ACT.Relu,
                                     bias=bias2_t[:, d2b:d2b + 1])
                nc.gpsimd.dma_start(
                    y[128 * d2b:128 * (d2b + 1), 128 * sb:128 * (sb + 1)],
                    y_t[:, d2b, :])

            def mk_out(sb):
                for d2b in range(4):
                    mk_out_block(sb, d2b)

            # interleave wst strips with window-0 strips in first-use order;
            # cs-outer MM emission for the first two windows (below) lets
            # mm1 start as soon as (wst[0], xw0 strip 0) land
            for s in range(4):
                nc.sync.dma_start(wst_t[s][:],
                                  wst[128 * s:128 * (s + 1), :])
                issue_xw_strip(0, s)
            for s in range(4):
                issue_xw_strip(1, s)
            issue_xw(2)
            issue_xw(3)
            at(2, load_consts)
            for c in range(NWIN + 9):
                if c < NWIN:
                    sb, j = divmod(c, 4)
                    if c + 4 < NWIN:
                        issue_xw(c + 4)
                    if j == 0:
                        state[sb] = {
                            "e": ep.tile([128, 4, 4, 512], F16, tag="e",
                                         name=f"e{sb}"),
                            "t": tp_.tile([128, 4, 4, 512], F16, tag="t",
                                          name=f"t{sb}"),
                        }
                        alloc_sums(sb)
                    st = state[sb]
                    xt = xw_t.pop(c)
                    last = c == NWIN - 1
                    pls = [psp.tile([128, 512], F32, tag="pl",
                                    name=f"pl{c}_{db}") for db in range(4)]
                    order = (
                        [(cs, db) for cs in range(4) for db in range(4)]
                        if c < 2 else
                        [(cs, db) for db in range(4) for cs in range(4)]
                    )
                    for oi, (cs, db) in enumerate(order):
                        nc.tensor.matmul(
                            pls[db][:],
                            wst_t[cs][:, 128 * db:128 * (db + 1)],
                            xt[:, cs, :],
                            start=(cs == 0), stop=(cs == 3),
                        )
                        if c < 2 and oi % 4 == 3 and oi < 12:
                            # bridge the next strip-DMA's landing so the PE
                            # stays continuously busy through the head ramp
                            warm_burst(3 - c)
                        if cs == 3:
                            nc.scalar.activation(st["e"][:, db, j, :],
                                                 pls[db][:], ACT.Exp)
                    nc.vector.tensor_mul(st["t"][:, :, j, :], xt[:],
                                         st["e"][:, :, j, :])
                    mk_tree_win(sb, j,
                                ("e", "t") if sb == NSB - 1 else ("e",))
                    if j == 3:
                        if sb < NSB - 1:
                            mk_tree_sb(sb, "t")
                        at(c + 1, lambda sb=sb: mk_recip(sb))
                        at(c + 2, lambda sb=sb: mk_pooled(sb))
                        at(c + 4, lambda sb=sb: mk_mm2(sb))
                        at(c + 5, lambda sb=sb: mk_out(sb))
                for fn in sched.pop(c, []):
                    fn()
    nc.compile()
    return nc


def _get_nc():
    if "nc" not in _cached:
        _cached["nc"] = _build()
    return _cached["nc"]


def _host_prep(x, w_score, w_conv, bn_gamma, bn_beta, bn_mean, bn_var):
    x = np.ascontiguousarray(np.asarray(x, dtype=np.float32)).reshape(
        B * N, K, C
    )
    w_score = np.asarray(w_score, dtype=np.float32)
    w_conv = np.asarray(w_conv, dtype=np.float32)
    inv = np.asarray(bn_gamma, dtype=np.float64) / np.sqrt(
        np.asarray(bn_var, dtype=np.float64) + BN_EPS
    )
    wc2 = w_conv.astype(np.float64) * inv[:, None]
    bias2 = (
        np.asarray(bn_beta, dtype=np.float64)
        - np.asarray(bn_mean, dtype=np.float64) * inv
    )
    common = {
        "wst": np.ascontiguousarray(w_score.T).astype(np.float16),
        "wc2": np.ascontiguousarray(wc2.T).astype(np.float16),
        # bias2[p, d2b] = BN bias for output channel 128*d2b + p
        "bias2": np.ascontiguousarray(
            bias2.reshape(4, 128).T.astype(np.float16)
        ),
    }
    x16 = x.astype(np.float16)
    in_maps = []
    for cr in range(NCORES):
        xc = x16[NROWS * cr:NROWS * (cr + 1)]          # [2048, 16, 512]
        # window layout [w, p, s, j]: j = k*32 + n2, channel = 128*s + p
        xwc = np.ascontiguousarray(
            xc.reshape(NWIN, 32, K, 4, 128).transpose(0, 4, 3, 2, 1)
        ).reshape(NWIN, 128, 4, 512)
        in_maps.append({"xw": xwc, **common})
    return in_maps


def kernel(x, w_score, w_conv, bn_gamma, bn_beta, bn_mean, bn_var):
    from concourse.bass_utils import run_bass_kernel_spmd

    nc = _get_nc()
    in_maps = _host_prep(x, w_score, w_conv, bn_gamma, bn_beta, bn_mean, bn_var)
    res = run_bass_kernel_spmd(nc, in_maps, core_ids=list(range(NCORES)))
    out = np.concatenate(
        [res.results[c]["y"].T for c in range(NCORES)], axis=0
    )
    return out.reshape(B, N, COUT).astype(np.float32)


# revision 33
# speedup vs baseline: 1.0012x; 1.0012x over previous
"""AttentionPooling TRN2 kernel v2: channel-on-partition layout, 8-core
data-parallel over the flattened (B*N) points.

Math (per point n with k=16 neighbors, C=512 channels):
  logits = x @ w_score.T            (per-channel attention logits)
  scores = softmax_k(logits)        (softmax over the k axis, per channel)
  pooled = sum_k x * scores
  y      = relu((pooled @ w_conv.T - mean) * gamma/sqrt(var+eps) + beta)

v1 kept (n,k) points on SBUF partitions, which forced the softmax-k
reductions onto the TensorE as G-matrix matmuls (~27us) and required PE
transposes before mm2 (~4us); PE busy was 296us of a 324us span.

v2 puts CHANNELS on partitions everywhere:
  - mm1 runs with the weights stationary (wstT [c,128d] blocks) and
    host-transposed fp16 xT as the moving operand, producing logits as
    [d-block, pts] in PSUM.  Same 218us of PE streaming, but the output
    layout now matches x's, so everything downstream is elementwise.
  - pts within a 512-pt window are ordered (k,n2): j = k*32 + n2.  The
    softmax-k reductions become log2(16)=4 pairwise tensor_tensor adds
    over contiguous half-slices -- fp16 2x_1P DVE mode, batched over all
    4 c-strips and 4 windows of a super-block in single instructions
    (58-cycle fixed cost amortized to nothing).
  - the whole k-reduction moves to the DVE (~131us) which was 60% idle,
    and mm2 consumes pooled [c,n] directly as its stationary operand --
    no transposes.  PE work drops to mm1 + mm2 + warmup = ~237us.
  - exp stays on ACT (146us), 1/esum uses the ACT Ln/Exp chain (exp is
    needed anyway and DVE reciprocal is 8.7ns/elem), relu stays on ACT.
    All three activations are pinned to act-table set 6 (which really
    holds exp+ln+relu) via _patch_act_tables: one table load per kernel
    instead of two 1.28us loads per super-block on the critical chain.
  - mm2 is TRANSPOSED: out [d2-block, n] with wc2 blocks stationary and
    pooled moving, so the BN bias becomes a per-partition ACT-relu bias
    (the K=1 bias matmul, a full 216ns stream slot, disappears) and the
    16 128-free MMs stream at the measured 55.8ns warm rate.  y is
    produced [COUT, NROWS] per core and untransposed on the host.
  - engine budgets: PE ~237us, DVE ~225us, ACT ~190us, DMA 32MB/core
    (~90us) -- vs v1's 64MB (x was shipped twice in two layouts).
  - fp16 everywhere (x, weights, e, t, sums): mm1 in fp16 is 8x more
    accurate than v1's bf16 (sim: 6.6e-4 vs 1.6e-3 end-to-end rel err).
    fp8 DoubleRow mm1 was evaluated and rejected: e4m3 on both operands
    measures 2.8e-2 end-to-end (gate 2e-2), and the pair slots cannot
    carry error-compensation terms without giving back the 2x.
  - emission schedule: window c emits 16 MMs (cs-outer for the first two
    windows so mm1 starts after 256KB of DMA), 4 exps, 1 mul, and the
    window's e-tree; each super-block's t-tree runs as 4 batched DVE ops
    after window 3's mul, and the per-sb tail (recip, pooled-mul, mm2,
    relu+store) is spread over the next 4 window slots so no engine
    stream stalls on a cross-engine dep.  The last super-block spreads
    its t-tree per window and fuses relu+store into mm2's d2-blocks to
    shorten the serial tail; dep-free warm-up matmul bursts bridge the
    head DMA ramp so the HAM clock-gate opens with the real stream.
  - measured: 276.5us (v1 baseline: 324.5us), rel err 7.9e-4; ~16us of
    span is fixed NEFF pre/postamble, mm1 streams at a gapless 216ns/MM.
"""
import numpy as np
import ml_dtypes

B, N, K, C, COUT = 4, 4096, 16, 512, 512
NCORES = 8
NROWS = B * N // NCORES           # 2048 n-points per core
NWIN = 64                         # 512-pt windows (32 n x 16 k) per core
NSB = 16                          # super-blocks of 4 windows (128 n)
BN_EPS = 1e-5

_cached = {}


def _patch_act_tables():
    """Pin Exp/Ln/Relu to the one table set that holds all three.

    The act-table-placement pass resolves each activation to a table set
    containing its function; exp's and ln's first-match sets differ, so the
    v2 profile showed 34 ACT_TABLE_LOADs (1.28us each) -- two per
    super-block around the Ln/Exp reciprocal, injected straight into the
    pooled->mm2 critical chain.  act_info.json's set 6
    (natural_log_exp_and_others) really contains exp+ln+relu; hiding those
    funcs from every OTHER set (list order, and therefore the positional
    act_func_set_id the pass emits, is unchanged) forces all three to
    resolve to set 6: one table load at kernel start, zero thereafter.
    The patch only narrows the pass's view; the runtime table actually
    loaded is the real set 6, so lowering stays correct.
    """
    import functools

    import concourse.bacc as bacc
    import concourse.hw_specs as hw_specs

    orig = hw_specs.get_activation_tables.__wrapped__

    @functools.cache
    def patched(arch):
        tabs = dict(orig(arch))
        keep = {"Exp", "Ln", "Relu"}
        out = {}
        for name, funcs in tabs.items():
            if name != "natural_log_exp_and_others":
                funcs = {f for f in funcs if f.name not in keep}
            out[name] = funcs
        return out

    bacc.get_activation_tables = patched


def _build():
    import concourse.bacc as bacc
    import concourse.mybir as mybir
    import concourse.tile as tile

    _patch_act_tables()
    F32, F16 = mybir.dt.float32, mybir.dt.float16
    ACT = mybir.ActivationFunctionType

    nc = bacc.Bacc("TRN2", target_bir_lowering=False, debug=False, num_devices=NCORES)
    xw = nc.dram_tensor("xw", [NWIN, 128, 4, 512], F16, kind="ExternalInput")
    wst = nc.dram_tensor("wst", [C, C], F16, kind="ExternalInput")      # w_score.T
    wc2 = nc.dram_tensor("wc2", [C, COUT], F16, kind="ExternalInput")   # (bn*w_conv).T
    bias2 = nc.dram_tensor("bias2", [128, 4], F16, kind="ExternalInput")
    y = nc.dram_tensor("y", [COUT, NROWS], F16, kind="ExternalOutput")

    with tile.TileContext(nc) as tc:
        with (
            tc.tile_pool(name="const", bufs=1) as cp,
            tc.tile_pool(name="xw", bufs=6) as xwp,
            tc.tile_pool(name="e", bufs=2) as ep,
            tc.tile_pool(name="t", bufs=2) as tp_,
            tc.tile_pool(name="r", bufs=1) as rp,
            tc.tile_pool(name="s", bufs=3) as sp,
            tc.tile_pool(name="o", bufs=2) as op_,
            tc.tile_pool(name="ps", bufs=8, space="PSUM") as psp,
        ):
            # ---- constants ----
            # wst strips interleave with window-0's strip DMAs below so the
            # first matmul is gated by 256KB of HBM traffic, not ~1MB; the
            # mm2-only constants (wc2/bias/ones) are deferred off the head's
            # HBM bandwidth entirely (first use is slot 7, ~25us in).
            wst_t = [cp.tile([128, C], F16, tag=f"wst{i}", name=f"wst{i}") for i in range(4)]
            wc2_t = [cp.tile([128, COUT], F16, tag=f"wc2{i}", name=f"wc2{i}") for i in range(4)]
            bias2_t = cp.tile([128, 4], F16, tag="bias2", name="bias2")

            def load_consts():
                for i in range(4):
                    nc.gpsimd.dma_start(wc2_t[i][:], wc2[128 * i:128 * (i + 1), :])
                nc.gpsimd.dma_start(bias2_t[:], bias2[:])

            # Dep-free warm-up: memset fp16 tile matmuls from t~0.3us keep the
            # PE busy while the first ~1MB of weights+x loads, so the HAM
            # clock-gate reaches 8/8 right as the real mm1 stream starts.
            wz0 = cp.tile([128, 128], F16, tag="wz0", name="wz0")
            nc.vector.memset(wz0[:], 0.25)
            warm_ps = psp.tile([128, C], F32, tag="pl", name="warm")
            import itertools
            warm_groups = itertools.cycle(range(4))

            def warm_burst(n):
                # dep-free matmuls on a memset tile: keep the PE's HAM
                # activity window busy through the head DMA ramp; each burst
                # is its own accumulation group on a distinct column slice
                g = next(warm_groups)
                for wmi in range(n):
                    nc.tensor.matmul(
                        warm_ps[:, 128 * g:128 * (g + 1)], wz0[:], wz0[:],
                        start=(wmi == 0), stop=(wmi == n - 1),
                    )

            warm_burst(24)
            # dep-free dummy Ln preloads the natural_log/exp ACT table set
            dume = cp.tile([1, 16], F32, tag="dume", name="dume")
            nc.vector.memset(dume[:], 1.0)
            dln = cp.tile([1, 16], F32, tag="dln", name="dln")
            nc.scalar.activation(dln[:], dume[:], ACT.Ln)

            xw_t = {}

            def issue_xw(w):
                t = xwp.tile([128, 4, 512], F16, tag="xw", name=f"xw{w}")
                nc.sync.dma_start(t[:], xw[w])
                xw_t[w] = t

            def issue_xw_strip(w, s):
                if w not in xw_t:
                    xw_t[w] = xwp.tile([128, 4, 512], F16, tag="xw",
                                       name=f"xw{w}")
                nc.sync.dma_start(xw_t[w][:, s, :], xw[w, :, s, :])

            state = {}
            sched = {}

            def at(slot, fn):
                sched.setdefault(slot, []).append(fn)

            def halve(dst, src, n):
                nc.vector.tensor_add(
                    dst[:], src[..., 0:n], src[..., n:2 * n]
                )

            def alloc_sums(sb):
                st = state[sb]
                st["es"] = sp.tile([128, 4, 4, 32], F16, tag="se",
                                   name=f"se{sb}")
                st["ts"] = sp.tile([128, 4, 4, 32], F16, tag="st",
                                   name=f"st{sb}")

            def mk_tree_sb(sb, key):
                # whole-super-block tree: 4 ops covering all 4 windows and
                # strips -- the 58-cycle DVE op overhead amortizes away
                st = state[sb]
                r1 = rp.tile([128, 4, 4, 256], F16, tag=f"r1{key}",
                             name=f"r1{key}{sb}")
                halve(r1, st[key], 256)
                r2 = rp.tile([128, 4, 4, 128], F16, tag=f"r2{key}",
                             name=f"r2{key}{sb}")
                halve(r2, r1, 128)
                r3 = rp.tile([128, 4, 4, 64], F16, tag=f"r3{key}",
                             name=f"r3{key}{sb}")
                halve(r3, r2, 64)
                halve(st["es" if key == "e" else "ts"], r3, 32)

            def mk_tree_win(sb, j, keys):
                # one window, all strips: spreads the reduction across the
                # window slots (e always; t only for the last super-block)
                st = state[sb]
                for key in keys:
                    src = st[key][:, :, j, :]
                    r1 = rp.tile([128, 4, 256], F16, tag=f"v1{key}",
                                 name=f"v1{key}{sb}_{j}")
                    nc.vector.tensor_add(r1[:], src[:, :, 0:256],
                                         src[:, :, 256:512])
                    r2 = rp.tile([128, 4, 128], F16, tag=f"v2{key}",
                                 name=f"v2{key}{sb}_{j}")
                    nc.vector.tensor_add(r2[:], r1[:, :, 0:128],
                                         r1[:, :, 128:256])
                    r3 = rp.tile([128, 4, 64], F16, tag=f"v3{key}",
                                 name=f"v3{key}{sb}_{j}")
                    nc.vector.tensor_add(r3[:], r2[:, :, 0:64],
                                         r2[:, :, 64:128])
                    dst = st["es" if key == "e" else "ts"]
                    nc.vector.tensor_add(dst[:, :, j, :], r3[:, :, 0:32],
                                         r3[:, :, 32:64])

            def mk_tree_strip(sb, j, db):
                # last-super-block tail: per-(window, strip) micro-trees so
                # strip db's sums complete while strip db+1's exp still runs
                st = state[sb]
                for key in ("e", "t"):
                    src = st[key][:, db, j, :]
                    r1 = rp.tile([128, 256], F16, tag=f"w1{key}",
                                 name=f"w1{key}{sb}_{j}_{db}")
                    nc.vector.tensor_add(r1[:], src[:, 0:256], src[:, 256:512])
                    r2 = rp.tile([128, 128], F16, tag=f"w2{key}",
                                 name=f"w2{key}{sb}_{j}_{db}")
                    nc.vector.tensor_add(r2[:], r1[:, 0:128], r1[:, 128:256])
                    r3 = rp.tile([128, 64], F16, tag=f"w3{key}",
                                 name=f"w3{key}{sb}_{j}_{db}")
                    nc.vector.tensor_add(r3[:], r2[:, 0:64], r2[:, 64:128])
                    dst = st["es" if key == "e" else "ts"]
                    nc.vector.tensor_add(dst[:, db, j, :], r3[:, 0:32],
                                         r3[:, 32:64])

            def mk_recip(sb):
                st = state[sb]
                lnt = sp.tile([128, 4, 4, 32], F32, tag="lnt", name=f"lnt{sb}")
                nc.scalar.activation(lnt[:], st["es"][:], ACT.Ln)
                inv = sp.tile([128, 4, 4, 32], F16, tag="inv", name=f"inv{sb}")
                nc.scalar.activation(inv[:], lnt[:], ACT.Exp, scale=-1.0)
                st["inv"] = inv

            def mk_pooled(sb):
                st = state[sb]
                pool_t = sp.tile([128, 4, 4, 32], F16, tag="pool",
                                 name=f"pool{sb}")
                nc.vector.tensor_mul(pool_t[:], st["ts"][:], st["inv"][:])
                st["pool"] = pool_t

            def mk_mm2(sb, fuse_out=False):
                # transposed mm2: out [d2-block, n] so the BN bias becomes a
                # per-partition ACT relu bias and the 216ns K=1 bias matmul
                # disappears; 16 MMs of 128-free stream at ~56ns each
                st = state[sb]
                py = psp.tile([128, 4, 128], F32, tag="pl", name=f"py{sb}")
                st["py"] = py
                for d2b in range(4):
                    for cs in range(4):
                        nc.tensor.matmul(
                            py[:, d2b, :],
                            wc2_t[cs][:, 128 * d2b:128 * (d2b + 1)],
                            st["pool"][:, cs],
                            start=(cs == 0), stop=(cs == 3),
                        )
                    if fuse_out:
                        # tail: overlap each d2-block's relu+store with the
                        # next block's matmuls
                        mk_out_block(sb, d2b)

            def mk_out_block(sb, d2b):
                st = state[sb]
                if d2b == 0:
                    st["yt"] = op_.tile([128, 4, 128], F16, tag="yt",
                                        name=f"yt{sb}")
                y_t = st["yt"]
                nc.scalar.activation(y_t[:, d2b, :], st["py"][:, d2b, :],
                                     ACT.Relu,
                                     bias=bias2_t[:, d2b:d2b + 1])
                nc.gpsimd.dma_start(
                    y[128 * d2b:128 * (d2b + 1), 128 * sb:128 * (sb + 1)],
                    y_t[:, d2b, :])

            def mk_out(sb):
                for d2b in range(4):
                    mk_out_block(sb, d2b)

            # interleave wst strips with window-0 strips in first-use order;
            # cs-outer MM emission for the first two windows (below) lets
            # mm1 start as soon as (wst[0], xw0 strip 0) land
            for s in range(4):
                nc.sync.dma_start(wst_t[s][:],
                                  wst[128 * s:128 * (s + 1), :])
                issue_xw_strip(0, s)
            for s in range(4):
                issue_xw_strip(1, s)
            issue_xw(2)
            issue_xw(3)
            at(2, load_consts)
            for c in range(NWIN + 9):
                if c < NWIN:
                    sb, j = divmod(c, 4)
                    if c + 4 < NWIN:
                        issue_xw(c + 4)
                    if j == 0:
                        state[sb] = {
                            "e": ep.tile([128, 4, 4, 512], F16, tag="e",
                                         name=f"e{sb}"),
                            "t": tp_.tile([128, 4, 4, 512], F16, tag="t",
                                          name=f"t{sb}"),
                        }
                        alloc_sums(sb)
                    st = state[sb]
                    xt = xw_t.pop(c)
                    last = c == NWIN - 1
                    pls = [psp.tile([128, 512], F32, tag="pl",
                                    name=f"pl{c}_{db}") for db in range(4)]
                    order = (
                        [(cs, db) for cs in range(4) for db in range(4)]
                        if c < 2 else
                        [(cs, db) for db in range(4) for cs in range(4)]
                    )
                    for oi, (cs, db) in enumerate(order):
                        nc.tensor.matmul(
                            pls[db][:],
                            wst_t[cs][:, 128 * db:128 * (db + 1)],
                            xt[:, cs, :],
                            start=(cs == 0), stop=(cs == 3),
                        )
                        if c < 2 and oi % 4 == 3 and oi < 12:
                            # bridge the next strip-DMA's landing so the PE
                            # stays continuously busy through the head ramp
                            warm_burst(3 - c)
                        if cs == 3:
                            nc.scalar.activation(st["e"][:, db, j, :],
                                                 pls[db][:], ACT.Exp)
                    nc.vector.tensor_mul(st["t"][:, :, j, :], xt[:],
                                         st["e"][:, :, j, :])
                    mk_tree_win(sb, j,
                                ("e", "t") if sb == NSB - 1 else ("e",))
                    if j == 3:
                        if sb < NSB - 1:
                            mk_tree_sb(sb, "t")
                        at(c + 1, lambda sb=sb: mk_recip(sb))
                        at(c + 2, lambda sb=sb: mk_pooled(sb))
                        at(c + 4, lambda sb=sb: mk_mm2(sb))
                        at(c + 5, lambda sb=sb: mk_out(sb))
                for fn in sched.pop(c, []):
                    fn()
    nc.compile()
    return nc


def _get_nc():
    if "nc" not in _cached:
        _cached["nc"] = _build()
    return _cached["nc"]


def _host_prep(x, w_score, w_conv, bn_gamma, bn_beta, bn_mean, bn_var):
    x = np.ascontiguousarray(np.asarray(x, dtype=np.float32)).reshape(
        B * N, K, C
    )
    w_score = np.asarray(w_score, dtype=np.float32)
    w_conv = np.asarray(w_conv, dtype=np.float32)
    inv = np.asarray(bn_gamma, dtype=np.float64) / np.sqrt(
        np.asarray(bn_var, dtype=np.float64) + BN_EPS
    )
    wc2 = w_conv.astype(np.float64) * inv[:, None]
    bias2 = (
        np.asarray(bn_beta, dtype=np.float64)
        - np.asarray(bn_mean, dtype=np.float64) * inv
    )
    common = {
        "wst": np.ascontiguousarray(w_score.T).astype(np.float16),
        "wc2": np.ascontiguousarray(wc2.T).astype(np.float16),
        # bias2[p, d2b] = BN bias for output channel 128*d2b + p
        "bias2": np.ascontiguousarray(
            bias2.reshape(4, 128).T.astype(np.float16)
        ),
    }
    x16 = x.astype(np.float16)
    in_maps = []
    for cr in range(NCORES):
        xc = x16[NROWS * cr:NROWS * (cr + 1)]          # [2048, 16, 512]
        # window layout [w, p, s, j]: j = k*32 + n2, channel = 128*s + p
        xwc = np.ascontiguousarray(
            xc.reshape(NWIN, 32, K, 4, 128).transpose(0, 4, 3, 2, 1)
        ).reshape(NWIN, 128, 4, 512)
        in_maps.append({"xw": xwc, **common})
    return in_maps


def kernel(x, w_score, w_conv, bn_gamma, bn_beta, bn_mean, bn_var):
    from concourse.bass_utils import run_bass_kernel_spmd

    nc = _get_nc()
    in_maps = _host_prep(x, w_score, w_conv, bn_gamma, bn_beta, bn_mean, bn_var)
    res = run_bass_kernel_spmd(nc, in_maps, core_ids=list(range(NCORES)))
    out = np.concatenate(
        [res.results[c]["y"].T for c in range(NCORES)], axis=0
    )
    return out.reshape(B, N, COUT).astype(np.float32)


# revision 34
# speedup vs baseline: 1.1926x; 1.1912x over previous
"""AttentionPooling TRN2 kernel v2: channel-on-partition layout, 8-core
data-parallel over the flattened (B*N) points.

Math (per point n with k=16 neighbors, C=512 channels):
  logits = x @ w_score.T            (per-channel attention logits)
  scores = softmax_k(logits)        (softmax over the k axis, per channel)
  pooled = sum_k x * scores
  y      = relu((pooled @ w_conv.T - mean) * gamma/sqrt(var+eps) + beta)

v1 kept (n,k) points on SBUF partitions, which forced the softmax-k
reductions onto the TensorE as G-matrix matmuls (~27us) and required PE
transposes before mm2 (~4us); PE busy was 296us of a 324us span.

v2 puts CHANNELS on partitions everywhere:
  - mm1 runs with the weights stationary (wstT [c,128d] blocks) and
    host-transposed fp16 xT as the moving operand, producing logits as
    [d-block, pts] in PSUM.  Same 218us of PE streaming, but the output
    layout now matches x's, so everything downstream is elementwise.
  - pts within a 512-pt window are ordered (k,n2): j = k*32 + n2.  The
    softmax-k reductions become log2(16)=4 pairwise tensor_tensor adds
    over contiguous half-slices -- fp16 2x_1P DVE mode, batched over all
    4 c-strips and 4 windows of a super-block in single instructions
    (58-cycle fixed cost amortized to nothing).
  - the whole k-reduction moves to the DVE (~131us) which was 60% idle,
    and mm2 consumes pooled [c,n] directly as its stationary operand --
    no transposes.  PE work drops to mm1 + mm2 + warmup = ~237us.
  - exp stays on ACT (146us), 1/esum uses the ACT Ln/Exp chain (exp is
    needed anyway and DVE reciprocal is 8.7ns/elem), relu stays on ACT.
    All three activations are pinned to act-table set 6 (which really
    holds exp+ln+relu) via _patch_act_tables: one table load per kernel
    instead of two 1.28us loads per super-block on the critical chain.
  - mm2 is TRANSPOSED: out [d2-block, n] with wc2 blocks stationary and
    pooled moving, so the BN bias becomes a per-partition ACT-relu bias
    (the K=1 bias matmul, a full 216ns stream slot, disappears) and the
    16 128-free MMs stream at the measured 55.8ns warm rate.  y is
    produced [COUT, NROWS] per core and untransposed on the host.
  - engine budgets: PE ~237us, DVE ~225us, ACT ~190us, DMA 32MB/core
    (~90us) -- vs v1's 64MB (x was shipped twice in two layouts).
  - fp16 everywhere (x, weights, e, t, sums): mm1 in fp16 is 8x more
    accurate than v1's bf16 (sim: 6.6e-4 vs 1.6e-3 end-to-end rel err).
    fp8 DoubleRow mm1 was evaluated and rejected: e4m3 on both operands
    measures 2.8e-2 end-to-end (gate 2e-2), and the pair slots cannot
    carry error-compensation terms without giving back the 2x.
  - emission schedule: window c emits 16 MMs (cs-outer for the first two
    windows so mm1 starts after 256KB of DMA), 4 exps, 1 mul, and the
    window's e-tree; each super-block's t-tree runs as 4 batched DVE ops
    after window 3's mul, and the per-sb tail (recip, pooled-mul, mm2,
    relu+store) is spread over the next 4 window slots so no engine
    stream stalls on a cross-engine dep.  The last super-block spreads
    its t-tree per window and fuses relu+store into mm2's d2-blocks to
    shorten the serial tail; dep-free warm-up matmul bursts bridge the
    head DMA ramp so the HAM clock-gate opens with the real stream.
  - measured: 276.5us (v1 baseline: 324.5us), rel err 7.9e-4; ~16us of
    span is fixed NEFF pre/postamble, mm1 streams at a gapless 216ns/MM.
"""
import numpy as np
import ml_dtypes

B, N, K, C, COUT = 4, 4096, 16, 512, 512
NCORES = 8
NROWS = B * N // NCORES           # 2048 n-points per core
NWIN = 64                         # 512-pt windows (32 n x 16 k) per core
NSB = 16                          # super-blocks of 4 windows (128 n)
BN_EPS = 1e-5

_cached = {}


def _patch_act_tables():
    """Pin Exp/Ln/Relu to the one table set that holds all three.

    The act-table-placement pass resolves each activation to a table set
    containing its function; exp's and ln's first-match sets differ, so the
    v2 profile showed 34 ACT_TABLE_LOADs (1.28us each) -- two per
    super-block around the Ln/Exp reciprocal, injected straight into the
    pooled->mm2 critical chain.  act_info.json's set 6
    (natural_log_exp_and_others) really contains exp+ln+relu; hiding those
    funcs from every OTHER set (list order, and therefore the positional
    act_func_set_id the pass emits, is unchanged) forces all three to
    resolve to set 6: one table load at kernel start, zero thereafter.
    The patch only narrows the pass's view; the runtime table actually
    loaded is the real set 6, so lowering stays correct.
    """
    import functools

    import concourse.bacc as bacc
    import concourse.hw_specs as hw_specs

    orig = hw_specs.get_activation_tables.__wrapped__

    @functools.cache
    def patched(arch):
        tabs = dict(orig(arch))
        combined = tabs.get("natural_log_exp_and_others")
        keep = {"Exp", "Ln", "Relu"}
        if combined is None or not keep <= {f.name for f in combined}:
            return tabs
        out = {}
        for name, funcs in tabs.items():
            if name != "natural_log_exp_and_others":
                funcs = {f for f in funcs if f.name not in keep}
            out[name] = funcs
        return out

    bacc.get_activation_tables = patched


def _build():
    import concourse.bacc as bacc
    import concourse.mybir as mybir
    import concourse.tile as tile

    _patch_act_tables()
    F32, F16 = mybir.dt.float32, mybir.dt.float16
    ACT = mybir.ActivationFunctionType

    nc = bacc.Bacc("TRN2", target_bir_lowering=False, debug=False, num_devices=NCORES)
    xw = nc.dram_tensor("xw", [NWIN, 128, 4, 512], F16, kind="ExternalInput")
    wst = nc.dram_tensor("wst", [C, C], F16, kind="ExternalInput")      # w_score.T
    wc2 = nc.dram_tensor("wc2", [C, COUT], F16, kind="ExternalInput")   # (bn*w_conv).T
    bias2 = nc.dram_tensor("bias2", [128, 4], F16, kind="ExternalInput")
    y = nc.dram_tensor("y", [COUT, NROWS], F16, kind="ExternalOutput")

    with tile.TileContext(nc) as tc:
        with (
            tc.tile_pool(name="const", bufs=1) as cp,
            tc.tile_pool(name="xw", bufs=6) as xwp,
            tc.tile_pool(name="e", bufs=2) as ep,
            tc.tile_pool(name="t", bufs=2) as tp_,
            tc.tile_pool(name="r", bufs=1) as rp,
            tc.tile_pool(name="s", bufs=3) as sp,
            tc.tile_pool(name="o", bufs=2) as op_,
            tc.tile_pool(name="ps", bufs=8, space="PSUM") as psp,
        ):
            # ---- constants ----
            # wst strips interleave with window-0's strip DMAs below so the
            # first matmul is gated by 256KB of HBM traffic, not ~1MB; the
            # mm2-only constants (wc2/bias/ones) are deferred off the head's
            # HBM bandwidth entirely (first use is slot 7, ~25us in).
            wst_t = [cp.tile([128, C], F16, tag=f"wst{i}", name=f"wst{i}") for i in range(4)]
            wc2_t = [cp.tile([128, COUT], F16, tag=f"wc2{i}", name=f"wc2{i}") for i in range(4)]
            bias2_t = cp.tile([128, 4], F16, tag="bias2", name="bias2")

            def load_consts():
                for i in range(4):
                    nc.gpsimd.dma_start(wc2_t[i][:], wc2[128 * i:128 * (i + 1), :])
                nc.gpsimd.dma_start(bias2_t[:], bias2[:])

            # Dep-free warm-up: memset fp16 tile matmuls from t~0.3us keep the
            # PE busy while the first ~1MB of weights+x loads, so the HAM
            # clock-gate reaches 8/8 right as the real mm1 stream starts.
            wz0 = cp.tile([128, 128], F16, tag="wz0", name="wz0")
            nc.vector.memset(wz0[:], 0.25)
            warm_ps = psp.tile([128, C], F32, tag="pl", name="warm")
            import itertools
            warm_groups = itertools.cycle(range(4))

            def warm_burst(n):
                # dep-free matmuls on a memset tile: keep the PE's HAM
                # activity window busy through the head DMA ramp; each burst
                # is its own accumulation group on a distinct column slice
                g = next(warm_groups)
                for wmi in range(n):
                    nc.tensor.matmul(
                        warm_ps[:, 128 * g:128 * (g + 1)], wz0[:], wz0[:],
                        start=(wmi == 0), stop=(wmi == n - 1),
                    )

            warm_burst(24)
            # dep-free dummy Ln preloads the natural_log/exp ACT table set
            dume = cp.tile([1, 16], F32, tag="dume", name="dume")
            nc.vector.memset(dume[:], 1.0)
            dln = cp.tile([1, 16], F32, tag="dln", name="dln")
            nc.scalar.activation(dln[:], dume[:], ACT.Ln)

            xw_t = {}

            def issue_xw(w):
                t = xwp.tile([128, 4, 512], F16, tag="xw", name=f"xw{w}")
                nc.sync.dma_start(t[:], xw[w])
                xw_t[w] = t

            def issue_xw_strip(w, s):
                if w not in xw_t:
                    xw_t[w] = xwp.tile([128, 4, 512], F16, tag="xw",
                                       name=f"xw{w}")
                nc.sync.dma_start(xw_t[w][:, s, :], xw[w, :, s, :])

            state = {}
            sched = {}

            def at(slot, fn):
                sched.setdefault(slot, []).append(fn)

            def halve(dst, src, n):
                nc.vector.tensor_add(
                    dst[:], src[..., 0:n], src[..., n:2 * n]
                )

            def alloc_sums(sb):
                st = state[sb]
                st["es"] = sp.tile([128, 4, 4, 32], F16, tag="se",
                                   name=f"se{sb}")
                st["ts"] = sp.tile([128, 4, 4, 32], F16, tag="st",
                                   name=f"st{sb}")

            def mk_tree_sb(sb, key):
                # whole-super-block tree: 4 ops covering all 4 windows and
                # strips -- the 58-cycle DVE op overhead amortizes away
                st = state[sb]
                r1 = rp.tile([128, 4, 4, 256], F16, tag=f"r1{key}",
                             name=f"r1{key}{sb}")
                halve(r1, st[key], 256)
                r2 = rp.tile([128, 4, 4, 128], F16, tag=f"r2{key}",
                             name=f"r2{key}{sb}")
                halve(r2, r1, 128)
                r3 = rp.tile([128, 4, 4, 64], F16, tag=f"r3{key}",
                             name=f"r3{key}{sb}")
                halve(r3, r2, 64)
                halve(st["es" if key == "e" else "ts"], r3, 32)

            def mk_tree_win(sb, j, keys):
                # one window, all strips: spreads the reduction across the
                # window slots (e always; t only for the last super-block)
                st = state[sb]
                for key in keys:
                    src = st[key][:, :, j, :]
                    r1 = rp.tile([128, 4, 256], F16, tag=f"v1{key}",
                                 name=f"v1{key}{sb}_{j}")
                    nc.vector.tensor_add(r1[:], src[:, :, 0:256],
                                         src[:, :, 256:512])
                    r2 = rp.tile([128, 4, 128], F16, tag=f"v2{key}",
                                 name=f"v2{key}{sb}_{j}")
                    nc.vector.tensor_add(r2[:], r1[:, :, 0:128],
                                         r1[:, :, 128:256])
                    r3 = rp.tile([128, 4, 64], F16, tag=f"v3{key}",
                                 name=f"v3{key}{sb}_{j}")
                    nc.vector.tensor_add(r3[:], r2[:, :, 0:64],
                                         r2[:, :, 64:128])
                    dst = st["es" if key == "e" else "ts"]
                    nc.vector.tensor_add(dst[:, :, j, :], r3[:, :, 0:32],
                                         r3[:, :, 32:64])

            def mk_tree_strip(sb, j, db):
                # last-super-block tail: per-(window, strip) micro-trees so
                # strip db's sums complete while strip db+1's exp still runs
                st = state[sb]
                for key in ("e", "t"):
                    src = st[key][:, db, j, :]
                    r1 = rp.tile([128, 256], F16, tag=f"w1{key}",
                                 name=f"w1{key}{sb}_{j}_{db}")
                    nc.vector.tensor_add(r1[:], src[:, 0:256], src[:, 256:512])
                    r2 = rp.tile([128, 128], F16, tag=f"w2{key}",
                                 name=f"w2{key}{sb}_{j}_{db}")
                    nc.vector.tensor_add(r2[:], r1[:, 0:128], r1[:, 128:256])
                    r3 = rp.tile([128, 64], F16, tag=f"w3{key}",
                                 name=f"w3{key}{sb}_{j}_{db}")
                    nc.vector.tensor_add(r3[:], r2[:, 0:64], r2[:, 64:128])
                    dst = st["es" if key == "e" else "ts"]
                    nc.vector.tensor_add(dst[:, db, j, :], r3[:, 0:32],
                                         r3[:, 32:64])

            def mk_recip(sb):
                st = state[sb]
                lnt = sp.tile([128, 4, 4, 32], F32, tag="lnt", name=f"lnt{sb}")
                nc.scalar.activation(lnt[:], st["es"][:], ACT.Ln)
                inv = sp.tile([128, 4, 4, 32], F16, tag="inv", name=f"inv{sb}")
                nc.scalar.activation(inv[:], lnt[:], ACT.Exp, scale=-1.0)
                st["inv"] = inv

            def mk_pooled(sb):
                st = state[sb]
                pool_t = sp.tile([128, 4, 4, 32], F16, tag="pool",
                                 name=f"pool{sb}")
                nc.vector.tensor_mul(pool_t[:], st["ts"][:], st["inv"][:])
                st["pool"] = pool_t

            def mk_mm2(sb, fuse_out=False):
                # transposed mm2: out [d2-block, n] so the BN bias becomes a
                # per-partition ACT relu bias and the 216ns K=1 bias matmul
                # disappears; 16 MMs of 128-free stream at ~56ns each
                st = state[sb]
                py = psp.tile([128, 4, 128], F32, tag="pl", name=f"py{sb}")
                st["py"] = py
                for d2b in range(4):
                    for cs in range(4):
                        nc.tensor.matmul(
                            py[:, d2b, :],
                            wc2_t[cs][:, 128 * d2b:128 * (d2b + 1)],
                            st["pool"][:, cs],
                            start=(cs == 0), stop=(cs == 3),
                        )
                    if fuse_out:
                        # tail: overlap each d2-block's relu+store with the
                        # next block's matmuls
                        mk_out_block(sb, d2b)

            def mk_out_block(sb, d2b):
                st = state[sb]
                if d2b == 0:
                    st["yt"] = op_.tile([128, 4, 128], F16, tag="yt",
                                        name=f"yt{sb}")
                y_t = st["yt"]
                nc.scalar.activation(y_t[:, d2b, :], st["py"][:, d2b, :],
                                     ACT.Relu,
                                     bias=bias2_t[:, d2b:d2b + 1])
                nc.gpsimd.dma_start(
                    y[128 * d2b:128 * (d2b + 1), 128 * sb:128 * (sb + 1)],
                    y_t[:, d2b, :])

            def mk_out(sb):
                for d2b in range(4):
                    mk_out_block(sb, d2b)

            # interleave wst strips with window-0 strips in first-use order;
            # cs-outer MM emission for the first two windows (below) lets
            # mm1 start as soon as (wst[0], xw0 strip 0) land
            for s in range(4):
                nc.sync.dma_start(wst_t[s][:],
                                  wst[128 * s:128 * (s + 1), :])
                issue_xw_strip(0, s)
            for s in range(4):
                issue_xw_strip(1, s)
            issue_xw(2)
            issue_xw(3)
            at(2, load_consts)
            for c in range(NWIN + 9):
                if c < NWIN:
                    sb, j = divmod(c, 4)
                    if c + 4 < NWIN:
                        issue_xw(c + 4)
                    if j == 0:
                        state[sb] = {
                            "e": ep.tile([128, 4, 4, 512], F16, tag="e",
                                         name=f"e{sb}"),
                            "t": tp_.tile([128, 4, 4, 512], F16, tag="t",
                                          name=f"t{sb}"),
                        }
                        alloc_sums(sb)
                    st = state[sb]
                    xt = xw_t.pop(c)
                    last = c == NWIN - 1
                    pls = [psp.tile([128, 512], F32, tag="pl",
                                    name=f"pl{c}_{db}") for db in range(4)]
                    order = (
                        [(cs, db) for cs in range(4) for db in range(4)]
                        if c < 2 else
                        [(cs, db) for db in range(4) for cs in range(4)]
                    )
                    for oi, (cs, db) in enumerate(order):
                        nc.tensor.matmul(
                            pls[db][:],
                            wst_t[cs][:, 128 * db:128 * (db + 1)],
                            xt[:, cs, :],
                            start=(cs == 0), stop=(cs == 3),
                        )
                        if c < 2 and oi % 4 == 3 and oi < 12:
                            # bridge the next strip-DMA's landing so the PE
                            # stays continuously busy through the head ramp
                            warm_burst(3 - c)
                        if cs == 3:
                            nc.scalar.activation(st["e"][:, db, j, :],
                                                 pls[db][:], ACT.Exp)
                    nc.vector.tensor_mul(st["t"][:, :, j, :], xt[:],
                                         st["e"][:, :, j, :])
                    mk_tree_win(sb, j,
                                ("e", "t") if sb == NSB - 1 else ("e",))
                    if j == 3:
                        if sb < NSB - 1:
                            mk_tree_sb(sb, "t")
                        at(c + 1, lambda sb=sb: mk_recip(sb))
                        at(c + 2, lambda sb=sb: mk_pooled(sb))
                        at(c + 4, lambda sb=sb: mk_mm2(sb))
                        at(c + 5, lambda sb=sb: mk_out(sb))
                for fn in sched.pop(c, []):
                    fn()
    nc.compile()
    return nc


def _get_nc():
    if "nc" not in _cached:
        _cached["nc"] = _build()
    return _cached["nc"]


def _host_prep(x, w_score, w_conv, bn_gamma, bn_beta, bn_mean, bn_var):
    x = np.ascontiguousarray(np.asarray(x, dtype=np.float32)).reshape(
        B * N, K, C
    )
    w_score = np.asarray(w_score, dtype=np.float32)
    w_conv = np.asarray(w_conv, dtype=np.float32)
    inv = np.asarray(bn_gamma, dtype=np.float64) / np.sqrt(
        np.asarray(bn_var, dtype=np.float64) + BN_EPS
    )
    wc2 = w_conv.astype(np.float64) * inv[:, None]
    bias2 = (
        np.asarray(bn_beta, dtype=np.float64)
        - np.asarray(bn_mean, dtype=np.float64) * inv
    )
    common = {
        "wst": np.ascontiguousarray(w_score.T).astype(np.float16),
        "wc2": np.ascontiguousarray(wc2.T).astype(np.float16),
        # bias2[p, d2b] = BN bias for output channel 128*d2b + p
        "bias2": np.ascontiguousarray(
            bias2.reshape(4, 128).T.astype(np.float16)
        ),
    }
    x16 = x.astype(np.float16)
    in_maps = []
    for cr in range(NCORES):
        xc = x16[NROWS * cr:NROWS * (cr + 1)]          # [2048, 16, 512]
        # window layout [w, p, s, j]: j = k*32 + n2, channel = 128*s + p
        xwc = np.ascontiguousarray(
            xc.reshape(NWIN, 32, K, 4, 128).transpose(0, 4, 3, 2, 1)
        ).reshape(NWIN, 128, 4, 512)
        in_maps.append({"xw": xwc, **common})
    return in_maps


def kernel(x, w_score, w_conv, bn_gamma, bn_beta, bn_mean, bn_var):
    from concourse.bass_utils import run_bass_kernel_spmd

    nc = _get_nc()
    in_maps = _host_prep(x, w_score, w_conv, bn_gamma, bn_beta, bn_mean, bn_var)
    res = run_bass_kernel_spmd(nc, in_maps, core_ids=list(range(NCORES)))
    out = np.concatenate(
        [res.results[c]["y"].T for c in range(NCORES)], axis=0
    )
    return out.reshape(B, N, COUT).astype(np.float32)
